# revision 1
# baseline (speedup 1.0000x reference)
"""AttentiveTransformer v3: fp16 pipeline, resident fT/WT, DMA-broadcast scales.

Per core: 8192 rows = 64 row-tiles of 128 (one ghost-BN chunk each), processed
in pairs for stats batching.
Host prep: center f per chunk (exact fp32), cast fp16, transpose -> fT.
Device per pair of tiles:
  PE : x~ = fcT.T @ WT (16 fp16 matmuls/tile, fp32 PSUM)
       + 8 selector-matmuls accumulating colsum(x~^2) onto PSUM partitions 0-7
  Act: xsq = Square(px) fp16, x16 = Copy(px) fp16, s8 = sqrt(vps/128 + eps)
  DVE: ab8 = 1/s8 (fp16); z = x16*pa (fp16 2x); top8; tau; out=max(z-tau,0) 4x
  DMA: gather ab8 [8,512] -> [1,4096] row; partition-broadcast row -> bc16
  Pool: pa = p16 * bc16 (fp16, SBUF)
Output fp16, host casts to fp32.
"""
import numpy as np

import concourse.bass as bass
import concourse.bacc as bacc
import concourse.tile as tile
from concourse import mybir
from concourse.bass_utils import run_bass_kernel_spmd

F32 = mybir.dt.float32
F16 = mybir.dt.float16
BN_EPS = 1e-5

B_FULL, IN, G = 65536, 512, 2048
N_CORES = 8
P = 128
NT = 4          # 4 n-tiles of 512 over G
KT = 4          # 4 k-tiles of 128 over IN
ACT = mybir.ActivationFunctionType
ALU = mybir.AluOpType


def build(n_tiles, gamma_trivial):
    from contextlib import ExitStack
    assert n_tiles % 2 == 0
    nc = bacc.Bacc()
    rows = n_tiles * P
    ft_d = nc.dram_tensor("ft", [IN, rows], F16, kind="ExternalInput")
    wt_d = nc.dram_tensor("wt", [IN, G], F16, kind="ExternalInput")
    p_d = nc.dram_tensor("p", [rows, G], F16, kind="ExternalInput")
    sel_d = nc.dram_tensor("sel64", [P, 64], F16, kind="ExternalInput")
    rinv_d = nc.dram_tensor("rinv8", [P, 16], F32, kind="ExternalInput")
    g8_d = nc.dram_tensor("g8", [8, 512], F16, kind="ExternalInput")
    out_d = nc.dram_tensor("out", [rows, G], F16, kind="ExternalOutput")
    abs_d = nc.dram_tensor("abscratch", [n_tiles // 2, 8 * 512], F16,
                           kind="Internal")

    with tile.TileContext(nc) as tc, ExitStack() as ctx:
        singles = ctx.enter_context(tc.tile_pool(name="singles", bufs=1))
        ppool = ctx.enter_context(tc.tile_pool(name="ppool", bufs=3))
        sqpool = ctx.enter_context(tc.tile_pool(name="sqpool", bufs=2))
        xpool = ctx.enter_context(tc.tile_pool(name="xpool", bufs=3))
        papool = ctx.enter_context(tc.tile_pool(name="papool", bufs=2))
        zpool = ctx.enter_context(tc.tile_pool(name="zpool", bufs=3))
        opool = ctx.enter_context(tc.tile_pool(name="opool", bufs=2))
        stpool = ctx.enter_context(tc.tile_pool(name="stpool", bufs=2))
        bcpool = ctx.enter_context(tc.tile_pool(name="bcpool", bufs=3))
        smpool = ctx.enter_context(tc.tile_pool(name="smpool", bufs=3))
        ps_x = ctx.enter_context(tc.tile_pool(name="ps_x", bufs=6, space="PSUM"))
        ps_v = ctx.enter_context(tc.tile_pool(name="ps_v", bufs=2, space="PSUM"))

        # ---- resident constants / weights / features ----
        ftk = []
        for k in range(KT):
            t = singles.tile([P, rows], F16, tag=f"ft{k}")
            nc.sync.dma_start(t[:], ft_d[k * P:(k + 1) * P, :])
            ftk.append(t)
        wtk = []
        for k in range(KT):
            t = singles.tile([P, G], F16, tag=f"wt{k}")
            nc.sync.dma_start(t[:], wt_d[k * P:(k + 1) * P, :])
            wtk.append(t)
        sel64 = singles.tile([P, 64], F16)
        nc.sync.dma_start(sel64[:], sel_d[:])
        rinv16 = singles.tile([P, 16], F32)
        nc.sync.dma_start(rinv16[:], rinv_d[:])
        zeros8 = singles.tile([P, 8], F16)
        nc.vector.memset(zeros8[:], 0.0)
        eps8 = singles.tile([8, 1], F32)
        nc.vector.memset(eps8[:], BN_EPS)
        g8 = singles.tile([8, 512], F16)
        if not gamma_trivial:
            nc.sync.dma_start(g8[:], g8_d[:])

        # ---- per-pair pipeline ----
        for cp in range(n_tiles // 2):
            tiles = (2 * cp, 2 * cp + 1)
            vb8 = ps_v.tile([8, 512], F32, tag="vb")
            p16s, x16s = {}, {}
            for ti, c in enumerate(tiles):
                p16 = ppool.tile([P, G], F16, tag=f"p16_{ti}")
                nc.sync.dma_start(p16[:], p_d[c * P:(c + 1) * P, :])
                p16s[ti] = p16
                for n in range(NT):
                    s = 4 * ti + n
                    px = ps_x.tile([P, 512], F32, tag="px")
                    for k in range(KT):
                        nc.tensor.matmul(px[:], ftk[k][:, c * P:(c + 1) * P],
                                         wtk[k][:, n * 512:(n + 1) * 512],
                                         start=(k == 0), stop=(k == KT - 1))
                    xsq = sqpool.tile([P, 512], F16, tag="sq")
                    nc.scalar.activation(xsq[:], px[:], ACT.Square)
                    x16 = xpool.tile([P, 512], F16, tag=f"x{ti}{n}")
                    nc.scalar.activation(x16[:], px[:], ACT.Copy)
                    x16s[(ti, n)] = x16
                    # accumulate colsum(xsq) onto PSUM partition s
                    nc.tensor.matmul(vb8[:], sel64[:, 8 * s:8 * (s + 1)],
                                     xsq[:], start=(s == 0), stop=(s == 7))

            s8 = stpool.tile([8, 512], F32, tag="s8")
            nc.scalar.activation(s8[:], vb8[:], ACT.Sqrt, bias=eps8[:],
                                 scale=1.0 / P)
            abf = stpool.tile([8, 512], F32, tag="abf")
            nc.vector.reciprocal_approx_fast(abf[:], s8[:])
            ab8 = stpool.tile([8, 512], F16, tag="ab8")
            nc.scalar.activation(ab8[:], abf[:], ACT.Copy)
            if not gamma_trivial:
                nc.gpsimd.tensor_tensor(ab8[:], ab8[:], g8[:], op=ALU.mult)
            nc.gpsimd.dma_start(abs_d[cp:cp + 1, :], ab8[:])

            cs2 = smpool.tile([P, 16], F32, tag="cs2")
            z16s = {}
            for ti, c in enumerate(tiles):
                bc16 = bcpool.tile([P, G], F16, tag=f"bc{ti}")
                nc.scalar.dma_start(
                    bc16[:],
                    abs_d[cp:cp + 1, ti * G:(ti + 1) * G].to_broadcast([P, G]))
                z16 = zpool.tile([P, G], F16, tag=f"z{ti}")
                for n in range(NT):
                    pa = papool.tile([P, 512], F16, tag=f"pa{n}")
                    nc.gpsimd.tensor_tensor(pa[:],
                                            p16s[ti][:, n * 512:(n + 1) * 512],
                                            bc16[:, n * 512:(n + 1) * 512],
                                            op=ALU.mult)
                    zeng = nc.gpsimd if (n == 3 and ti == 0) else nc.vector
                    zeng.tensor_tensor(z16[:, n * 512:(n + 1) * 512],
                                       x16s[(ti, n)][:], pa[:], op=ALU.mult)

                m8 = smpool.tile([P, 8], F16, tag=f"m8_{ti}")
                nc.vector.max(m8[:], z16[:])
                nc.vector.tensor_tensor_scan(cs2[:, 8 * ti:8 * (ti + 1)],
                                             m8[:], zeros8[:], 0.0,
                                             op0=ALU.add, op1=ALU.bypass)
                z16s[ti] = z16
            taur2 = smpool.tile([P, 16], F32, tag="tr2")
            nc.vector.scalar_tensor_tensor(taur2[:], in0=cs2[:], scalar=-1.0,
                                           in1=rinv16[:], op0=ALU.add,
                                           op1=ALU.mult)
            for ti, c in enumerate(tiles):
                ntau = smpool.tile([P, 1], F32, tag=f"nt{ti}")
                nc.vector.tensor_reduce(ntau[:],
                                        taur2[:, 8 * ti:8 * (ti + 1)],
                                        axis=mybir.AxisListType.X,
                                        op=ALU.max, negate=True)
                nc.vector.tensor_scalar(out=z16s[ti][:], in0=z16s[ti][:],
                                        scalar1=ntau[:], scalar2=0.0,
                                        op0=ALU.add, op1=ALU.max)
                nc.sync.dma_start(out_d[c * P:(c + 1) * P, :], z16s[ti][:])

    nc.finalize()
    return nc


_CACHE = {}


def _host_prep(priors, feat, W, gamma, n_cores, shard):
    # center per ghost chunk in fp32, cast fp16
    fc = feat.reshape(-1, P, IN)
    fc = fc - fc.mean(axis=1, keepdims=True)
    fc16 = fc.astype(np.float16).reshape(-1, IN)
    fT = np.ascontiguousarray(fc16.T)              # [IN, B] fp16
    wt = np.ascontiguousarray(W.T.astype(np.float16))  # [IN, G]
    p16 = priors.astype(np.float16)
    sel64 = np.zeros((P, 64), np.float16)
    for s in range(8):
        sel64[:, 8 * s + s] = 1.0
    rinv8 = np.broadcast_to(np.tile(1.0 / np.arange(1, 9, dtype=np.float32), 2), (P, 16)).copy()
    g8 = np.ascontiguousarray(
        np.tile(gamma.reshape(4, 512), (2, 1)).astype(np.float16))
    return fT, wt, p16, sel64, rinv8, g8


def kernel(priors, processed_feat, W, gamma, beta):
    priors = np.ascontiguousarray(priors, dtype=np.float32)
    feat = np.ascontiguousarray(processed_feat, dtype=np.float32)
    W = np.ascontiguousarray(W, dtype=np.float32)
    gamma = np.asarray(gamma, dtype=np.float32)
    beta = np.asarray(beta, dtype=np.float32)
    assert bool(np.all(beta == 0.0)), "beta != 0 path not implemented"
    gamma_trivial = bool(np.all(gamma == 1.0))

    B = feat.shape[0]
    shard = B // N_CORES
    n_tiles = shard // P

    key = (n_tiles, gamma_trivial)
    if key not in _CACHE:
        _CACHE[key] = build(*key)
    nc = _CACHE[key]

    fT, wt, p16, sel64, rinv8, g8 = _host_prep(priors, feat, W, gamma,
                                               N_CORES, shard)
    in_maps = []
    for i in range(N_CORES):
        in_maps.append({
            "ft": np.ascontiguousarray(fT[:, i * shard:(i + 1) * shard]),
            "wt": wt,
            "p": p16[i * shard:(i + 1) * shard],
            "sel64": sel64,
            "rinv8": rinv8,
            "g8": g8,
        })
    res = run_bass_kernel_spmd(nc, in_maps, core_ids=list(range(N_CORES)))
    return np.concatenate([r["out"].astype(np.float32) for r in res.results],
                          axis=0)

